# revision 1
# baseline (speedup 1.0000x reference)
"""CollaborativeAttention Trainium2 kernel.

Sharding: 8 cores = (batch b in {0,1}) x (512-query-row block). Each core
computes its 512 output rows end to end; k/v/content-bias are computed
redundantly within each batch group, so no device collectives are needed.

Per-core math (R=512 own rows, S=2048 keys, D=1024, H=16 heads):
  qT[c,i]   = sum_d WqT[d,c] x[i,d]          (c on partitions)
  kT[c,j]   = sum_d WkT[d,c] x[j,d]
  v[j,hv]   = sum_d x[j,d] WvT[d,hv]         (+ ones column per head)
  cbT[j,h]  = sum_d x[j,d] WcbT[d,h]/8
  scoresT_h[j,i] = sum_c kT[c,j] * (qT[c,i]*m[h,c]/8)
  probsu_h  = exp(scoresT_h + cbT[:,h])       (no max-sub: scores are O(1))
  ctxT[hv,i], den[h,i] via probs matmul against [v_h | 1]
  out = LN(x + (ctxT/den) @ WdT + bd')       bd' = bd + Wd @ bv
The j (key) axis is processed in 4 quarters of 512 to bound SBUF.
Each core's own query rows are permuted to j-columns 0:512 on the host, so
the SPMD program always projects q from quarter 0 (softmax is j-order
invariant as long as k/v/cb share the permutation, which they do).
"""

import sys

if '/opt/trn_rl_repo' not in sys.path:
    sys.path.insert(0, '/opt/trn_rl_repo')

import numpy as np

_CACHE = {}

B, S, D, H = 2, 2048, 1024, 16
R = 512          # query rows per core
DT = 8           # 128-partition tiles along d (contraction)
CT = 8           # 128-col chunks along c (=DK)
NQ = 4           # j quarters
JT = 4           # 128-j-tiles per quarter
HV = 64          # head dim


SCORES_FP8 = True  # fp8e4m3 DoubleRow for the scores matmul (2x PE throughput)


def _build():
    import concourse.bass as bass
    from concourse import bacc
    import concourse.mybir as mybir
    import concourse.tile as tile

    f32 = mybir.dt.float32
    f32r = mybir.dt.float32r
    f8 = mybir.dt.float8e4
    AF = mybir.ActivationFunctionType
    ALU = mybir.AluOpType
    DR = mybir.MatmulPerfMode.DoubleRow

    nc = bacc.Bacc("TRN2", debug=False, target_bir_lowering=False)

    xT_d = nc.dram_tensor("xT", [D, S], f32r, kind="ExternalInput").ap()
    xr_d = nc.dram_tensor("xrows", [R, D], f32, kind="ExternalInput").ap()
    wq_d = nc.dram_tensor("wq", [D, D], f32r, kind="ExternalInput").ap()
    wk_d = nc.dram_tensor("wk", [D, D], f32r, kind="ExternalInput").ap()
    wv_d = nc.dram_tensor("wv", [D, D], f32r, kind="ExternalInput").ap()
    wcb_d = nc.dram_tensor("wcb", [D, H], f32r, kind="ExternalInput").ap()
    mt_d = nc.dram_tensor("mt", [D, H], f32, kind="ExternalInput").ap()
    wd_d = nc.dram_tensor("wd", [D, D], f32r, kind="ExternalInput").ap()
    bde_d = nc.dram_tensor("bde", [1, D], f32, kind="ExternalInput").ap()
    gam_d = nc.dram_tensor("gamma2", [1, D], f32, kind="ExternalInput").ap()
    bet_d = nc.dram_tensor("beta2", [1, D], f32, kind="ExternalInput").ap()
    out_d = nc.dram_tensor("out", [R, D], f32, kind="ExternalOutput").ap()

    xTr = xT_d.rearrange("(dt p) j -> p dt j", p=128)
    wqr = wq_d.rearrange("(dt p) c -> p dt c", p=128)
    wkr = wk_d.rearrange("(dt p) c -> p dt c", p=128)
    wvr = wv_d.rearrange("(dt p) c -> p dt c", p=128)
    wdr = wd_d.rearrange("(dt p) c -> p dt c", p=128)
    wcbr = wcb_d.rearrange("(dt p) h -> p dt h", p=128)
    mtr = mt_d.rearrange("(dt p) h -> p dt h", p=128)

    def bcast_row(ap_row, n):
        return bass.AP(tensor=ap_row.tensor, offset=ap_row.offset,
                       ap=[[0, 128]] + [list(x) for x in ap_row.ap[1:]]) \
            if False else bass.AP(tensor=ap_row.tensor, offset=ap_row.offset,
                                  ap=[[0, 128], [1, n]])

    with tile.TileContext(nc) as tc:
        with tc.tile_pool(name="sp", bufs=1) as sp, \
             tc.tile_pool(name="pp", bufs=1, space="PSUM") as pp, \
             tc.tile_pool(name="dp", bufs=1, space="DRAM") as dp:

            den_dram = dp.tile([NQ, H, R], f32, tag="dend")
            rec_dram = dp.tile([H, R], f32, tag="recd")

            # resident constants
            mt_sb = sp.tile([128, DT, H], f32, tag="mt")
            nc.sync.dma_start(out=mt_sb, in_=mtr)
            wcb_sb = sp.tile([128, DT, H], f32r, tag="wcbt")
            nc.sync.dma_start(out=wcb_sb, in_=wcbr)
            bdeB = sp.tile([128, D], f32, tag="bdeB")
            nc.sync.dma_start(out=bdeB, in_=bcast_row(bde_d[0:1, :], D))
            gamB = sp.tile([128, D], f32, tag="gamB")
            nc.sync.dma_start(out=gamB, in_=bcast_row(gam_d[0:1, :], D))
            betB = sp.tile([128, D], f32, tag="betB")
            nc.sync.dma_start(out=betB, in_=bcast_row(bet_d[0:1, :], D))
            epsT = sp.tile([128, 1], f32, tag="epsT")
            nc.vector.memset(epsT, 1e-5)
            onesT = sp.tile([128, H, 1], f32, tag="onesT")
            nc.vector.memset(onesT, 1.0)

            qT = sp.tile([128, CT, R], f32, tag="qT")
            ctxu = sp.tile([128, 8, R], f32r, tag="ctxu")

            for q in range(NQ):
                xh = []
                for ch in range(2):
                    t = sp.tile([128, DT, 256], f32r, tag="xh", bufs=2,
                                name=f"xh_{q}_{ch}")
                    nc.sync.dma_start(
                        out=t, in_=xTr[:, :, q * 512 + ch * 256: q * 512 + (ch + 1) * 256])
                    xh.append(t)

                if q == 0:
                    # q projection from quarter 0 (own rows, host-permuted)
                    for ct in range(CT):
                        wqc = sp.tile([128, DT, 128], f32r, tag="wck", bufs=3,
                                      name=f"wqc_{ct}")
                        nc.sync.dma_start(out=wqc, in_=wqr[:, :, ct * 128:(ct + 1) * 128])
                        for ch in range(2):
                            ps = pp.tile([128, 256], f32, tag="ps", bufs=4,
                                         name=f"qps_{ct}_{ch}")
                            for dt in range(DT):
                                nc.tensor.matmul(ps, wqc[:, dt, :], xh[ch][:, dt, :],
                                                 start=(dt == 0), stop=(dt == DT - 1))
                            nc.vector.tensor_copy(
                                out=qT[:, ct, ch * 256:(ch + 1) * 256], in_=ps)

                # kT projection for this quarter
                if SCORES_FP8:
                    kT = sp.tile([128, 4, 2, 512], f8, tag="kT8", name=f"kT8_{q}")
                else:
                    kT = sp.tile([128, CT, 512], f32r, tag="kT", name=f"kT_{q}")
                for ct in range(CT):
                    wkc = sp.tile([128, DT, 128], f32r, tag="wck", bufs=3,
                                  name=f"wkc_{q}_{ct}")
                    nc.sync.dma_start(out=wkc, in_=wkr[:, :, ct * 128:(ct + 1) * 128])
                    for ch in range(2):
                        ps = pp.tile([128, 256], f32, tag="ps", bufs=4,
                                     name=f"kps_{q}_{ct}_{ch}")
                        for dt in range(DT):
                            nc.tensor.matmul(ps, wkc[:, dt, :], xh[ch][:, dt, :],
                                             start=(dt == 0), stop=(dt == DT - 1))
                        if SCORES_FP8:
                            nc.vector.tensor_copy(
                                out=kT[:, ct // 2, ct % 2, ch * 256:(ch + 1) * 256],
                                in_=ps)
                        else:
                            nc.vector.tensor_copy(
                                out=kT[:, ct, ch * 256:(ch + 1) * 256], in_=ps)

                # v projection (+ per-head ones column at 65h+64)
                vA = sp.tile([128, JT, H * 65], f32r, tag="vA", name=f"vA_{q}")
                for jt in range(JT):
                    ev = vA[:, jt, :].rearrange("p (h u) -> p h u", u=65)
                    nc.vector.tensor_copy(out=ev[:, :, 64:65], in_=onesT)
                for vc in range(4):
                    wvc = sp.tile([128, DT, 256], f32r, tag="wcv", bufs=2,
                                  name=f"wvc_{q}_{vc}")
                    nc.sync.dma_start(out=wvc, in_=wvr[:, :, vc * 256:(vc + 1) * 256])
                    for jt in range(JT):
                        ps = pp.tile([128, 256], f32, tag="ps", bufs=4,
                                     name=f"vps_{q}_{vc}_{jt}")
                        xcol = xh[jt // 2][:, :, (jt % 2) * 128:(jt % 2) * 128 + 128]
                        for dt in range(DT):
                            nc.tensor.matmul(ps, xcol[:, dt, :], wvc[:, dt, :],
                                             start=(dt == 0), stop=(dt == DT - 1))
                        ev = vA[:, jt, :].rearrange("p (h u) -> p h u", u=65)
                        nc.vector.tensor_copy(
                            out=ev[:, 4 * vc:4 * vc + 4, 0:64],
                            in_=ps.rearrange("p (h u) -> p h u", u=64))

                # content bias (pre-scaled by 1/8 on host)
                cbT = sp.tile([128, JT, H], f32, tag="cbT", bufs=2, name=f"cbT_{q}")
                for jt in range(JT):
                    ps16 = pp.tile([128, H], f32, tag="ps", bufs=4,
                                   name=f"cps16_{q}_{jt}")
                    xcol = xh[jt // 2][:, :, (jt % 2) * 128:(jt % 2) * 128 + 128]
                    for dt in range(DT):
                        nc.tensor.matmul(ps16, xcol[:, dt, :], wcb_sb[:, dt, :],
                                         start=(dt == 0), stop=(dt == DT - 1))
                    nc.vector.tensor_copy(out=cbT[:, jt, :], in_=ps16)

                # heads
                for h in range(H):
                    qms = []
                    if SCORES_FP8:
                        for g in range(4):
                            qm = sp.tile([128, 2, R], f8, tag="qm", bufs=6,
                                         name=f"qm_{q}_{h}_{g}")
                            for s2 in range(2):
                                nc.vector.tensor_scalar_mul(
                                    qm[:, s2, :], qT[:, 2 * g + s2, :],
                                    mt_sb[:, 2 * g + s2, h:h + 1])
                            qms.append(qm)
                    else:
                        for ct in range(CT):
                            qm = sp.tile([128, R], f32r, tag="qm", bufs=8,
                                         name=f"qm_{q}_{h}_{ct}")
                            nc.vector.tensor_scalar_mul(qm, qT[:, ct, :],
                                                        mt_sb[:, ct, h:h + 1])
                            qms.append(qm)
                    prbs = []
                    for jt in range(JT):
                        ps = pp.tile([128, R], f32, tag="ps", bufs=4,
                                     name=f"sps_{q}_{h}_{jt}")
                        if SCORES_FP8:
                            for g in range(4):
                                nc.tensor.matmul(
                                    ps, kT[:, g, :, jt * 128:(jt + 1) * 128],
                                    qms[g], start=(g == 0), stop=(g == 3),
                                    perf_mode=DR)
                        else:
                            for ct in range(CT):
                                nc.tensor.matmul(
                                    ps, kT[:, ct, jt * 128:(jt + 1) * 128],
                                    qms[ct], start=(ct == 0), stop=(ct == CT - 1))
                        pr = sp.tile([128, R], f32r, tag="pr", bufs=4,
                                     name=f"pr_{q}_{h}_{jt}")
                        nc.scalar.activation(out=pr, in_=ps, func=AF.Exp,
                                             bias=cbT[:, jt, h:h + 1],
                                             scale=(1.0 / 64.0 if SCORES_FP8 else 1.0))
                        prbs.append(pr)
                    cps = pp.tile([65, R], f32, tag="cps", bufs=2,
                                  name=f"cps_{q}_{h}")
                    for jt in range(JT):
                        nc.tensor.matmul(cps, vA[:, jt, h * 65:h * 65 + 65],
                                         prbs[jt], start=(jt == 0),
                                         stop=(jt == JT - 1))
                    pb = (h % 2) * 64
                    sub = h // 2
                    if q == 0:
                        nc.vector.tensor_copy(out=ctxu[pb:pb + 64, sub, :],
                                              in_=cps[0:64, :])
                    else:
                        nc.vector.tensor_tensor(out=ctxu[pb:pb + 64, sub, :],
                                                in0=cps[0:64, :],
                                                in1=ctxu[pb:pb + 64, sub, :],
                                                op=ALU.add)
                    stg = sp.tile([65, R], f32, tag="stg", bufs=2,
                                  name=f"stg_{q}_{h}")
                    nc.vector.tensor_copy(out=stg[64:65, :], in_=cps[64:65, :])
                    nc.sync.dma_start(out=den_dram[q, h:h + 1, :],
                                      in_=stg[64:65, :])

            # denominator: sum quarters, reciprocal, stage via DRAM
            dacc = sp.tile([H, R], f32, tag="dsA", name="dacc0")
            nc.sync.dma_start(out=dacc, in_=den_dram[0])
            for q in range(1, NQ):
                dq = sp.tile([H, R], f32, tag="den", bufs=2, name=f"dq_{q}")
                nc.sync.dma_start(out=dq, in_=den_dram[q])
                dnew = sp.tile([H, R], f32, tag=("dsB" if q % 2 else "dsA"),
                               name=f"dacc{q}")
                nc.vector.tensor_tensor(out=dnew, in0=dacc, in1=dq, op=ALU.add)
                dacc = dnew
            rec = sp.tile([H, R], f32, tag="dsA", name="rec")
            nc.vector.reciprocal(out=rec, in_=dacc)
            nc.sync.dma_start(out=rec_dram, in_=rec)

            # normalize ctx (reuses the kT slot)
            ctxn = sp.tile([128, 8, R], f32r, tag="kT", name="ctxn")
            for s in range(8):
                rb = sp.tile([128, R], f32, tag="rb", bufs=2, name=f"rb_{s}")
                src = bass.AP(tensor=rec_dram.tensor,
                              offset=rec_dram[2 * s:2 * s + 2, :].offset,
                              ap=[[R, 2], [0, 64], [1, R]])
                nc.sync.dma_start(out=rb, in_=src)
                nc.vector.tensor_tensor(out=ctxn[:, s, :], in0=ctxu[:, s, :],
                                        in1=rb, op=ALU.mult)

            # dense + residual + layernorm
            res = [sp.tile([128, D], f32, tag="res", bufs=4, name=f"res_{ic}")
                   for ic in range(4)]
            for oc in range(4):
                wdc = sp.tile([128, DT, 256], f32r, tag="wcv", bufs=2,
                              name=f"wdc_{oc}")
                nc.sync.dma_start(out=wdc, in_=wdr[:, :, oc * 256:(oc + 1) * 256])
                for ic in range(4):
                    ps = pp.tile([128, 256], f32, tag="ps", bufs=4,
                                 name=f"dps_{oc}_{ic}")
                    for vt in range(8):
                        nc.tensor.matmul(ps, ctxn[:, vt, ic * 128:(ic + 1) * 128],
                                         wdc[:, vt, :], start=(vt == 0),
                                         stop=(vt == 7))
                    xrc = sp.tile([128, 256], f32, tag="xrc", bufs=3,
                                  name=f"xrc_{oc}_{ic}")
                    nc.sync.dma_start(
                        out=xrc,
                        in_=xr_d[ic * 128:(ic + 1) * 128, oc * 256:(oc + 1) * 256])
                    nc.vector.tensor_tensor(
                        out=res[ic][:, oc * 256:(oc + 1) * 256], in0=ps, in1=xrc,
                        op=ALU.add)
            for ic in range(4):
                lnA = sp.tile([128, D], f32, tag="lnA", bufs=2, name=f"lnA_{ic}")
                nc.vector.tensor_tensor(out=lnA, in0=res[ic], in1=bdeB, op=ALU.add)
                stats = sp.tile([128, 2, nc.vector.BN_STATS_DIM], f32, tag="stats",
                                bufs=2, name=f"stats_{ic}")
                for g in range(2):
                    nc.vector.bn_stats(out=stats[:, g, :],
                                       in_=lnA[:, g * 512:(g + 1) * 512])
                mv = sp.tile([128, nc.vector.BN_AGGR_DIM], f32, tag="mv", bufs=2,
                             name=f"mv_{ic}")
                nc.vector.bn_aggr(out=mv, in_=stats)
                rstd = sp.tile([128, 1], f32, tag="rstd", bufs=2, name=f"rstd_{ic}")
                nc.scalar.activation(out=rstd, in_=mv[:, 1:2], func=AF.Sqrt,
                                     bias=epsT, scale=1.0)
                nc.vector.reciprocal(out=rstd, in_=rstd)
                nc.vector.tensor_scalar(out=res[ic], in0=lnA, scalar1=mv[:, 0:1],
                                        scalar2=rstd, op0=ALU.subtract,
                                        op1=ALU.mult)
                nc.vector.tensor_tensor(out=lnA, in0=res[ic], in1=gamB, op=ALU.mult)
                nc.vector.tensor_tensor(out=res[ic], in0=lnA, in1=betB, op=ALU.add)
                nc.sync.dma_start(out=out_d[ic * 128:(ic + 1) * 128, :], in_=res[ic])

    nc.compile()
    return nc


def _prep_in_maps(inputs):
    f = np.float32
    x = np.ascontiguousarray(np.asarray(inputs["hidden_states"], f))
    Wq = np.asarray(inputs["Wq"], f)
    Wk = np.asarray(inputs["Wk"], f)
    Wcb = np.asarray(inputs["Wcb"], f)
    Wv = np.asarray(inputs["Wv"], f)
    bv = np.asarray(inputs["bv"], f)
    mixing = np.asarray(inputs["mixing"], f)
    Wd = np.asarray(inputs["Wd"], f)
    bd = np.asarray(inputs["bd"], f)
    gamma = np.asarray(inputs["gamma"], f)
    beta = np.asarray(inputs["beta"], f)

    shared = {
        "wq": np.ascontiguousarray(Wq.T),
        "wk": np.ascontiguousarray(Wk.T),
        "wv": np.ascontiguousarray(Wv.T),
        "wcb": np.ascontiguousarray(Wcb.T / 8.0),
        "mt": np.ascontiguousarray(mixing.T * (8.0 if SCORES_FP8 else 0.125)),
        "wd": np.ascontiguousarray(Wd.T),
        "bde": np.ascontiguousarray((bd + Wd @ bv)[None, :]),
        "gamma2": np.ascontiguousarray(gamma[None, :]),
        "beta2": np.ascontiguousarray(beta[None, :]),
    }
    in_maps = []
    for c in range(8):
        b, rb = divmod(c, 4)
        r0 = rb * R
        xT = x[b].T
        cols = np.r_[r0:r0 + R, 0:r0, r0 + R:S]
        in_maps.append({
            "xT": np.ascontiguousarray(xT[:, cols]),
            "xrows": np.ascontiguousarray(x[b, r0:r0 + R]),
            **shared,
        })
    return in_maps


def _gather(results):
    out = np.empty((B, S, D), np.float32)
    for c in range(8):
        b, rb = divmod(c, 4)
        out[b, rb * R:(rb + 1) * R] = results[c]["out"]
    return out


def kernel(**inputs):
    from concourse.bass_utils import run_bass_kernel_spmd

    if "nc" not in _CACHE:
        _CACHE["nc"] = _build()
    nc = _CACHE["nc"]
    in_maps = _prep_in_maps(inputs)
    res = run_bass_kernel_spmd(nc, in_maps, core_ids=list(range(8)))
    return (_gather(res.results),)



# revision 16
# speedup vs baseline: 1.5661x; 1.5661x over previous
"""CollaborativeAttention Trainium2 kernel (v2, all-fp8 DoubleRow).

Sharding: 8 cores = (batch b in {0,1}) x (512-query-row block). Each core
computes its 512 output rows end to end; k/v/content-bias are computed
redundantly within each batch group, so no device collectives are needed.
Each core's own query rows are permuted to j-columns 0:512 on the host, so
the SPMD program always projects q from quarter 0 (softmax is j-order
invariant as long as k/v/cb share the permutation, which they do).

Precision: the attention contribution to the output is tiny relative to the
residual (std ~0.007 vs 1.0), so every matmul runs in fp8e4m3 DoubleRow
(2x PE throughput) with scales folded on the host:
  x -> fp8 directly; Wq,Wk,Wv,Wd -> fp8 x16 (keeps them out of the fp8
  subnormal range); Wcb -> fp8 x2; mixing -> f32 /2.
  scores_psum = (8 q.m)(16 k) = 1024*(q.m.k/8)  -> exp scale 1/1024
  cb_psum = 2*x.Wcb -> cbT = psum/16 = cb/8 (the exp bias)
  ctx_psum/den = 16*ctx -> fp8 ctxn; dense_psum = 256*(ctx@Wd)
  residual handled at 256x: xr2 = 256*(x + bd + Wd@bv); layernorm is
  scale-invariant given eps' = 256^2 * 1e-5.

Per-core DR matmul layout convention: contraction index c = g*256 + ko*128 + p
with p the partition, stationary [128, 2(ko), cols], moving [128, 2(ko), free].

Dataflow per quarter (j-block of 512 keys):
  kT8[c,j] (fp8, 32 MMs) ; vA[j,(h,65)] + cb (fp8, 48 MMs) ;
  for head-groups of 4: scores (4 banks) -> exp(+cb bias) -> pr fp8 ->
  ctx DR into [65,512] banks (row 64 = ones column = softmax denominator),
  drained into SBUF accumulator ctxu[65,16,512].
Tail: den -> DRAM -> reciprocal -> broadcast; ctxn fp8 = ctxu*rec;
  dense DR + residual + LN.
"""

import sys

if '/opt/trn_rl_repo' not in sys.path:
    sys.path.insert(0, '/opt/trn_rl_repo')

import numpy as np

_CACHE = {}

B, S, D, H = 2, 2048, 1024, 16
R = 512          # query rows per core
NQ = 4           # j quarters


def _build():
    import concourse.bass as bass
    from concourse import bacc
    import concourse.mybir as mybir
    import concourse.tile as tile

    f32 = mybir.dt.float32
    bf16 = mybir.dt.bfloat16
    f8 = mybir.dt.float8e4
    AF = mybir.ActivationFunctionType
    ALU = mybir.AluOpType
    DR = mybir.MatmulPerfMode.DoubleRow

    nc = bacc.Bacc("TRN2", debug=False, target_bir_lowering=False)

    xt8_d = nc.dram_tensor("xt8", [128, 4, 2, S], f8, kind="ExternalInput").ap()
    wq8_d = nc.dram_tensor("wq8", [128, 4, 2, D], f8, kind="ExternalInput").ap()
    wk8_d = nc.dram_tensor("wk8", [128, 4, 2, D], f8, kind="ExternalInput").ap()
    wv8_d = nc.dram_tensor("wv8", [128, 4, 2, D], f8, kind="ExternalInput").ap()
    wd8_d = nc.dram_tensor("wd8", [64, 8, 2, D], f8, kind="ExternalInput").ap()
    wcb8_d = nc.dram_tensor("wcb8", [128, 4, 2, H], f8, kind="ExternalInput").ap()
    mt_d = nc.dram_tensor("mt", [128, 8, H], f32, kind="ExternalInput").ap()
    xr2_d = nc.dram_tensor("xr2", [R, D], f32, kind="ExternalInput").ap()
    gam_d = nc.dram_tensor("gamma2", [1, D], bf16, kind="ExternalInput").ap()
    bet_d = nc.dram_tensor("beta2", [1, D], bf16, kind="ExternalInput").ap()
    out_d = nc.dram_tensor("out", [R, D], f32, kind="ExternalOutput").ap()

    def bcast_row(ap_row, n):
        return bass.AP(tensor=ap_row.tensor, offset=ap_row.offset,
                       ap=[[0, 128], [1, n]])

    with tile.TileContext(nc) as tc:
        with tc.tile_pool(name="sp", bufs=1) as sp, \
             tc.tile_pool(name="pp", bufs=1, space="PSUM") as pp, \
             tc.tile_pool(name="dp", bufs=1, space="DRAM") as dp:

            den_dram = dp.tile([1, H, R], bf16, tag="dend")
            rec_dram = dp.tile([H, R], f32, tag="recd")

            # resident inputs
            xt8 = sp.tile([128, 4, 2, S], f8, tag="xt8")
            nc.sync.dma_start(out=xt8, in_=xt8_d)
            wq8 = sp.tile([128, 4, 2, D], f8, tag="wq8")
            nc.sync.dma_start(out=wq8, in_=wq8_d)
            wk8 = sp.tile([128, 4, 2, D], f8, tag="wk8")
            nc.sync.dma_start(out=wk8, in_=wk8_d)
            wv8 = sp.tile([128, 4, 2, D], f8, tag="wv8")
            nc.sync.dma_start(out=wv8, in_=wv8_d)
            wd8 = sp.tile([64, 8, 2, D], f8, tag="wd8")
            nc.sync.dma_start(out=wd8, in_=wd8_d)
            wcb8 = sp.tile([128, 4, 2, H], f8, tag="wcb8")
            nc.sync.dma_start(out=wcb8, in_=wcb8_d)
            mt_sb = sp.tile([128, 8, H], f32, tag="mt")
            nc.sync.dma_start(out=mt_sb, in_=mt_d)
            gamB = sp.tile([128, D], bf16, tag="gamB")
            nc.sync.dma_start(out=gamB, in_=bcast_row(gam_d[0:1, :], D))
            betB = sp.tile([128, D], bf16, tag="betB")
            nc.sync.dma_start(out=betB, in_=bcast_row(bet_d[0:1, :], D))
            epsT = sp.tile([128, 1], f32, tag="epsT")
            nc.vector.memset(epsT, 1e-5 * 256.0 * 256.0)

            # persistent intermediates
            qT = sp.tile([128, 8, R], bf16, tag="qT")
            qm = sp.tile([128, H, 4, 2, R], f8, tag="qm")
            ctxu = sp.tile([65, H, R], bf16, tag="ctxu")
            ctxn = sp.tile([64, 8, 2, R], f8, tag="ctxn")

            # q projection (own rows = j 0:512), psum = 16*q
            for cb2 in range(8):
                ps = pp.tile([128, R], f32, tag="A", bufs=4, name=f"qps_{cb2}")
                for dg in range(4):
                    nc.tensor.matmul(ps, wq8[:, dg, :, cb2 * 128:(cb2 + 1) * 128],
                                     xt8[:, dg, :, 0:R],
                                     start=(dg == 0), stop=(dg == 3),
                                     perf_mode=DR)
                nc.scalar.copy(out=qT[:, cb2, :], in_=ps)

            # qm for head-group 0 on the scalar engine (early window)
            for h in range(4):
                for g in range(4):
                    for ko in range(2):
                        nc.scalar.mul(out=qm[:, h, g, ko, :],
                                      in_=qT[:, 2 * g + ko, :],
                                      mul=mt_sb[:, 2 * g + ko, h:h + 1])

            for q in range(NQ):
                jq = slice(q * R, (q + 1) * R)

                # k projection for this quarter -> fp8 (16*k)
                kT8 = sp.tile([128, 4, 2, R], f8, tag="kT8", bufs=2,
                              name=f"kT8_{q}")
                for cb2 in range(8):
                    ps = pp.tile([128, R], f32, tag="A", bufs=4,
                                 name=f"kps_{q}_{cb2}")
                    for dg in range(4):
                        nc.tensor.matmul(ps, wk8[:, dg, :, cb2 * 128:(cb2 + 1) * 128],
                                         xt8[:, dg, :, jq],
                                         start=(dg == 0), stop=(dg == 3),
                                         perf_mode=DR)
                    nc.vector.tensor_copy(out=kT8[:, cb2 // 2, cb2 % 2, :], in_=ps)

                # v projection (+ ones column) and content bias
                vA = sp.tile([128, 2, 2, H * 65], f8, tag="vA", bufs=2,
                             name=f"vA_{q}")
                cbT = sp.tile([128, 4, H], f32, tag="cbT", bufs=2, name=f"cbT_{q}")
                for jtp in range(2):
                    for ko in range(2):
                        ev = vA[:, jtp, ko, :].rearrange("p (h u) -> p h u", u=65)
                        nc.vector.memset(ev[:, :, 64:65], 1.0)
                for jt in range(4):
                    jb = slice(q * R + jt * 128, q * R + jt * 128 + 128)
                    psv = [pp.tile([128, R], f32, tag="A", bufs=4,
                                   name=f"vps_{q}_{jt}_{half}")
                           for half in range(2)]
                    pscb = pp.tile([128, H], f32, tag="C", bufs=4,
                                   name=f"cbps_{q}_{jt}")
                    for dg in range(4):
                        for half in range(2):
                            nc.tensor.matmul(psv[half], xt8[:, dg, :, jb],
                                             wv8[:, dg, :, half * 512:(half + 1) * 512],
                                             start=(dg == 0), stop=(dg == 3),
                                             perf_mode=DR)
                        nc.tensor.matmul(pscb, xt8[:, dg, :, jb], wcb8[:, dg, :, :],
                                         start=(dg == 0), stop=(dg == 3),
                                         perf_mode=DR)
                    ev = vA[:, jt // 2, jt % 2, :].rearrange("p (h u) -> p h u", u=65)
                    for half in range(2):
                        nc.vector.tensor_copy(
                            out=ev[:, half * 8:(half + 1) * 8, 0:64],
                            in_=psv[half].rearrange("p (h u) -> p h u", u=64))
                    nc.vector.tensor_scalar(out=cbT[:, jt, :], in0=pscb,
                                            scalar1=1.0 / 16.0, scalar2=None,
                                            op0=ALU.mult)

                if q == 0:
                    # remaining qm (DVE), after quarter-0 casts in program order
                    for h in range(4, H):
                        for g in range(4):
                            for ko in range(2):
                                nc.vector.tensor_scalar_mul(
                                    qm[:, h, g, ko, :], qT[:, 2 * g + ko, :],
                                    mt_sb[:, 2 * g + ko, h:h + 1])

                # attention: head groups of 4
                for hg in range(4):
                    prs = []
                    for hh in range(4):
                        pr = sp.tile([128, 2, 2, R], f8, tag="pr", bufs=4,
                                     name=f"pr_{q}_{hg}_{hh}")
                        prs.append(pr)
                    for jt in range(4):
                        pss = []
                        for hh in range(4):
                            pss.append(pp.tile([128, R], f32, tag="A", bufs=4,
                                               name=f"sps_{q}_{hg}_{jt}_{hh}"))
                        for g in range(4):
                            for hh in range(4):
                                h = hg * 4 + hh
                                nc.tensor.matmul(
                                    pss[hh], kT8[:, g, :, jt * 128:(jt + 1) * 128],
                                    qm[:, h, g, :, :],
                                    start=(g == 0), stop=(g == 3), perf_mode=DR)
                        for hh in range(4):
                            h = hg * 4 + hh
                            nc.scalar.activation(
                                out=prs[hh][:, jt // 2, jt % 2, :], in_=pss[hh],
                                func=AF.Exp, bias=cbT[:, jt, h:h + 1],
                                scale=1.0 / 1024.0)
                    for hh in range(4):
                        h = hg * 4 + hh
                        cps = pp.tile([65, R], f32, tag="C", bufs=4,
                                      name=f"cps_{q}_{hg}_{hh}")
                        for jtp in range(2):
                            nc.tensor.matmul(cps, vA[:, jtp, :, h * 65:h * 65 + 65],
                                             prs[hh][:, jtp, :, :],
                                             start=(jtp == 0), stop=(jtp == 1),
                                             perf_mode=DR)
                        if q == 0:
                            nc.vector.tensor_copy(out=ctxu[:, h, :], in_=cps)
                        else:
                            nc.vector.tensor_tensor(out=ctxu[:, h, :], in0=cps,
                                                    in1=ctxu[:, h, :], op=ALU.add)

            # denominator -> reciprocal (via DRAM to transpose [1,H,R]->[H,R])
            nc.sync.dma_start(out=den_dram, in_=ctxu[64:65, :, :])
            dl = sp.tile([H, R], bf16, tag="dl")
            dsrc = bass.AP(tensor=den_dram.tensor, offset=den_dram.offset,
                           ap=[[R, H], [1, R]])
            nc.sync.dma_start(out=dl, in_=dsrc)
            rec = sp.tile([H, R], f32, tag="rec")
            nc.vector.reciprocal(out=rec, in_=dl)
            nc.sync.dma_start(out=rec_dram, in_=rec)

            # normalize: ctxn[u, h//2, h%2, :] = ctxu[u, h, :] * rec[h, :]
            for h in range(H):
                rb = sp.tile([64, R], f32, tag="rb", bufs=2, name=f"rb_{h}")
                src = bass.AP(tensor=rec_dram.tensor,
                              offset=rec_dram[h:h + 1, :].offset,
                              ap=[[0, 64], [1, R]])
                nc.sync.dma_start(out=rb, in_=src)
                nc.vector.tensor_tensor(
                    out=ctxn[:, h // 2, h % 2, :],
                    in0=ctxu[0:64, h, :], in1=rb, op=ALU.mult)

            # dense (psum = 256*(ctx@Wd)) + residual (xr2 = 256*(x+bd')) + LN
            for ic in range(4):
                res = sp.tile([128, D], f32, tag="res", bufs=1, name=f"res_{ic}")
                xrc = sp.tile([128, D], f32, tag="xrc", bufs=1, name=f"xrc_{ic}")
                nc.sync.dma_start(out=xrc, in_=xr2_d[ic * 128:(ic + 1) * 128, :])
                for oh in range(2):
                    ps = pp.tile([128, 512], f32, tag="A", bufs=4,
                                 name=f"dps_{ic}_{oh}")
                    for s in range(8):
                        nc.tensor.matmul(ps, ctxn[:, s, :, ic * 128:(ic + 1) * 128],
                                         wd8[:, s, :, oh * 512:(oh + 1) * 512],
                                         start=(s == 0), stop=(s == 7),
                                         perf_mode=DR)
                    nc.vector.tensor_tensor(
                        out=res[:, oh * 512:(oh + 1) * 512], in0=ps,
                        in1=xrc[:, oh * 512:(oh + 1) * 512], op=ALU.add)
                stats = sp.tile([128, 2, nc.vector.BN_STATS_DIM], f32, tag="stats",
                                bufs=2, name=f"stats_{ic}")
                for g in range(2):
                    nc.vector.bn_stats(out=stats[:, g, :],
                                       in_=res[:, g * 512:(g + 1) * 512])
                mv = sp.tile([128, nc.vector.BN_AGGR_DIM], f32, tag="mv", bufs=2,
                             name=f"mv_{ic}")
                nc.vector.bn_aggr(out=mv, in_=stats)
                rstd = sp.tile([128, 1], f32, tag="rstd", bufs=2, name=f"rstd_{ic}")
                nc.scalar.activation(out=rstd, in_=mv[:, 1:2], func=AF.Sqrt,
                                     bias=epsT, scale=1.0)
                nc.vector.reciprocal(out=rstd, in_=rstd)
                lnA = sp.tile([128, D], f32, tag="lnA", bufs=2, name=f"lnA_{ic}")
                nc.vector.tensor_scalar(out=lnA, in0=res, scalar1=mv[:, 0:1],
                                        scalar2=rstd, op0=ALU.subtract,
                                        op1=ALU.mult)
                nc.vector.tensor_tensor(out=res, in0=lnA, in1=gamB, op=ALU.mult)
                nc.vector.tensor_tensor(out=lnA, in0=res, in1=betB, op=ALU.add)
                nc.sync.dma_start(out=out_d[ic * 128:(ic + 1) * 128, :], in_=lnA)

    nc.compile()
    return nc


def _arr8(mat, scale):
    """[Drows, C] f32 -> [128, 4, 2, C] fp8 with rows d = dg*256 + ko*128 + p."""
    import ml_dtypes
    a = (mat * scale).astype(ml_dtypes.float8_e4m3)
    C = a.shape[1]
    return np.ascontiguousarray(a.reshape(4, 2, 128, C).transpose(2, 0, 1, 3))


def _prep_in_maps(inputs):
    import ml_dtypes
    f = np.float32
    x = np.ascontiguousarray(np.asarray(inputs["hidden_states"], f))
    Wq = np.asarray(inputs["Wq"], f)
    Wk = np.asarray(inputs["Wk"], f)
    Wcb = np.asarray(inputs["Wcb"], f)
    Wv = np.asarray(inputs["Wv"], f)
    bv = np.asarray(inputs["bv"], f)
    mixing = np.asarray(inputs["mixing"], f)
    Wd = np.asarray(inputs["Wd"], f)
    bd = np.asarray(inputs["bd"], f)
    gamma = np.asarray(inputs["gamma"], f)
    beta = np.asarray(inputs["beta"], f)

    bde = (bd + Wd @ bv).astype(f)
    mt = np.ascontiguousarray(
        (mixing.T * 0.5).reshape(8, 128, H).transpose(1, 0, 2)).astype(f)
    shared = {
        "wq8": _arr8(Wq.T, 16.0),
        "wk8": _arr8(Wk.T, 16.0),
        "wv8": _arr8(Wv.T, 16.0),
        # dense stationary convention: v = (2s+ko)*64 + u -> [u, s, ko, o]
        "wd8": np.ascontiguousarray(
            (Wd.T * 16.0).astype(ml_dtypes.float8_e4m3)
            .reshape(8, 2, 64, D).transpose(2, 0, 1, 3)),
        "wcb8": _arr8(Wcb.T, 2.0),
        "mt": mt,
        "gamma2": np.ascontiguousarray(gamma[None, :]).astype(ml_dtypes.bfloat16),
        "beta2": np.ascontiguousarray(beta[None, :]).astype(ml_dtypes.bfloat16),
    }
    xb8 = [np.asarray(x[b].T, ml_dtypes.float8_e4m3) for b in range(B)]
    in_maps = []
    for c in range(8):
        b, rb = divmod(c, 4)
        r0 = rb * R
        cols = np.r_[r0:r0 + R, 0:r0, r0 + R:S]
        xp = xb8[b][:, cols]
        xt8 = np.ascontiguousarray(xp.reshape(4, 2, 128, S).transpose(2, 0, 1, 3))
        xr2 = np.ascontiguousarray(256.0 * (x[b, r0:r0 + R] + bde[None, :]))
        in_maps.append({"xt8": xt8, "xr2": xr2, **shared})
    return in_maps


def _gather(results):
    out = np.empty((B, S, D), np.float32)
    for c in range(8):
        b, rb = divmod(c, 4)
        out[b, rb * R:(rb + 1) * R] = results[c]["out"]
    return out


def kernel(**inputs):
    from concourse.bass_utils import run_bass_kernel_spmd

    if "nc" not in _CACHE:
        _CACHE["nc"] = _build()
    nc = _CACHE["nc"]
    in_maps = _prep_in_maps(inputs)
    res = run_bass_kernel_spmd(nc, in_maps, core_ids=list(range(8)))
    return (_gather(res.results),)


# revision 19
# speedup vs baseline: 1.6278x; 1.0394x over previous
"""CollaborativeAttention Trainium2 kernel (v2, all-fp8 DoubleRow).

Sharding: 8 cores = (batch b in {0,1}) x (512-query-row block). Each core
computes its 512 output rows end to end; k/v/content-bias are computed
redundantly within each batch group, so no device collectives are needed.
Each core's own query rows are permuted to j-columns 0:512 on the host, so
the SPMD program always projects q from quarter 0 (softmax is j-order
invariant as long as k/v/cb share the permutation, which they do).

Precision: the attention contribution to the output is tiny relative to the
residual (std ~0.007 vs 1.0), so every matmul runs in fp8e4m3 DoubleRow
(2x PE throughput) with scales folded on the host:
  x -> fp8 directly; Wq,Wk,Wv,Wd -> fp8 x16 (keeps them out of the fp8
  subnormal range); Wcb -> fp8 x2; mixing -> f32 /2.
  scores_psum = (8 q.m)(16 k) = 1024*(q.m.k/8)  -> exp scale 1/1024
  cb_psum = 2*x.Wcb -> cbT = psum/16 = cb/8 (the exp bias)
  ctx_psum/den = 16*ctx -> fp8 ctxn; dense_psum = 256*(ctx@Wd)
  residual handled at 256x: xr2 = 256*(x + bd + Wd@bv); layernorm is
  scale-invariant given eps' = 256^2 * 1e-5.

Per-core DR matmul layout convention: contraction index c = g*256 + ko*128 + p
with p the partition, stationary [128, 2(ko), cols], moving [128, 2(ko), free].

Dataflow per quarter (j-block of 512 keys):
  kT8[c,j] (fp8, 32 MMs) ; vA[j,(h,65)] + cb (fp8, 48 MMs) ;
  for head-groups of 4: scores (4 banks) -> exp(+cb bias) -> pr fp8 ->
  ctx DR into [65,512] banks (row 64 = ones column = softmax denominator),
  drained into SBUF accumulator ctxu[65,16,512].
Tail: den -> DRAM -> reciprocal -> broadcast; ctxn fp8 = ctxu*rec;
  dense DR + residual + LN.
"""

import sys

if '/opt/trn_rl_repo' not in sys.path:
    sys.path.insert(0, '/opt/trn_rl_repo')

import numpy as np

_CACHE = {}

B, S, D, H = 2, 2048, 1024, 16
R = 512          # query rows per core
NQ = 4           # j quarters


def _build():
    import concourse.bass as bass
    from concourse import bacc
    import concourse.mybir as mybir
    import concourse.tile as tile

    f32 = mybir.dt.float32
    bf16 = mybir.dt.bfloat16
    f8 = mybir.dt.float8e4
    AF = mybir.ActivationFunctionType
    ALU = mybir.AluOpType
    DR = mybir.MatmulPerfMode.DoubleRow

    nc = bacc.Bacc("TRN2", debug=False, target_bir_lowering=False)

    xt8_d = nc.dram_tensor("xt8", [128, 4, 2, S], f8, kind="ExternalInput").ap()
    wq8_d = nc.dram_tensor("wq8", [128, 4, 2, D], f8, kind="ExternalInput").ap()
    wk8_d = nc.dram_tensor("wk8", [128, 4, 2, D], f8, kind="ExternalInput").ap()
    wv8_d = nc.dram_tensor("wv8", [128, 4, 2, D], f8, kind="ExternalInput").ap()
    wd8_d = nc.dram_tensor("wd8", [64, 8, 2, D], f8, kind="ExternalInput").ap()
    wcb8_d = nc.dram_tensor("wcb8", [128, 4, 2, H], f8, kind="ExternalInput").ap()
    mt_d = nc.dram_tensor("mt", [128, 8, H], f32, kind="ExternalInput").ap()
    xr2_d = nc.dram_tensor("xr2", [R, D], f32, kind="ExternalInput").ap()
    gam_d = nc.dram_tensor("gamma2", [1, D], bf16, kind="ExternalInput").ap()
    bet_d = nc.dram_tensor("beta2", [1, D], bf16, kind="ExternalInput").ap()
    out_d = nc.dram_tensor("out", [R, D], f32, kind="ExternalOutput").ap()

    def bcast_row(ap_row, n):
        return bass.AP(tensor=ap_row.tensor, offset=ap_row.offset,
                       ap=[[0, 128], [1, n]])

    with tile.TileContext(nc) as tc:
        with tc.tile_pool(name="sp", bufs=1) as sp, \
             tc.tile_pool(name="pp", bufs=1, space="PSUM") as pp, \
             tc.tile_pool(name="dp", bufs=1, space="DRAM") as dp:

            den_dram = dp.tile([1, H, R], bf16, tag="dend")
            rec_dram = dp.tile([H, R], f32, tag="recd")

            # resident inputs (xt8 split so quarter 0 lands first)
            xt8 = sp.tile([128, 4, 2, S], f8, tag="xt8")
            nc.sync.dma_start(out=xt8[:, :, :, 0:R], in_=xt8_d[:, :, :, 0:R])
            wq8 = sp.tile([128, 4, 2, D], f8, tag="wq8")
            nc.sync.dma_start(out=wq8, in_=wq8_d)
            wk8 = sp.tile([128, 4, 2, D], f8, tag="wk8")
            nc.sync.dma_start(out=wk8, in_=wk8_d)
            for qq in range(1, 4):
                nc.sync.dma_start(out=xt8[:, :, :, qq * R:(qq + 1) * R],
                                  in_=xt8_d[:, :, :, qq * R:(qq + 1) * R])
            wv8 = sp.tile([128, 4, 2, D], f8, tag="wv8")
            nc.sync.dma_start(out=wv8, in_=wv8_d)
            wd8 = sp.tile([64, 8, 2, D], f8, tag="wd8")
            nc.sync.dma_start(out=wd8, in_=wd8_d)
            wcb8 = sp.tile([128, 4, 2, H], f8, tag="wcb8")
            nc.sync.dma_start(out=wcb8, in_=wcb8_d)
            mt_sb = sp.tile([128, 8, H], f32, tag="mt")
            nc.sync.dma_start(out=mt_sb, in_=mt_d)
            gamB = sp.tile([128, D], bf16, tag="gamB")
            nc.sync.dma_start(out=gamB, in_=bcast_row(gam_d[0:1, :], D))
            betB = sp.tile([128, D], bf16, tag="betB")
            nc.sync.dma_start(out=betB, in_=bcast_row(bet_d[0:1, :], D))
            epsT = sp.tile([128, 1], f32, tag="epsT")
            nc.vector.memset(epsT, 1e-5 * 256.0 * 256.0)

            # persistent intermediates
            qT = sp.tile([128, 8, R], bf16, tag="qT")
            qm = sp.tile([128, H, 4, 2, R], f8, tag="qm")
            ctxu = sp.tile([65, H, R], bf16, tag="ctxu")
            ctxn = sp.tile([64, 8, 2, R], f8, tag="ctxn")

            # q projection (own rows = j 0:512), psum = 16*q
            for cb2 in range(8):
                ps = pp.tile([128, R], f32, tag="A", bufs=5, name=f"qps_{cb2}")
                for dg in range(4):
                    nc.tensor.matmul(ps, wq8[:, dg, :, cb2 * 128:(cb2 + 1) * 128],
                                     xt8[:, dg, :, 0:R],
                                     start=(dg == 0), stop=(dg == 3),
                                     perf_mode=DR)
                nc.scalar.copy(out=qT[:, cb2, :], in_=ps)

            # qm for head-group 0 on the scalar engine (early window)
            for h in range(4):
                for g in range(4):
                    for ko in range(2):
                        nc.scalar.mul(out=qm[:, h, g, ko, :],
                                      in_=qT[:, 2 * g + ko, :],
                                      mul=mt_sb[:, 2 * g + ko, h:h + 1])

            for q in range(NQ):
                jq = slice(q * R, (q + 1) * R)

                # k projection for this quarter -> fp8 (16*k)
                kT8 = sp.tile([128, 4, 2, R], f8, tag="kT8", bufs=2,
                              name=f"kT8_{q}")
                for cb2 in range(8):
                    ps = pp.tile([128, R], f32, tag="A", bufs=5,
                                 name=f"kps_{q}_{cb2}")
                    for dg in range(4):
                        nc.tensor.matmul(ps, wk8[:, dg, :, cb2 * 128:(cb2 + 1) * 128],
                                         xt8[:, dg, :, jq],
                                         start=(dg == 0), stop=(dg == 3),
                                         perf_mode=DR)
                    nc.vector.tensor_copy(out=kT8[:, cb2 // 2, cb2 % 2, :], in_=ps)

                # v projection (+ ones column) and content bias
                vA = sp.tile([128, 2, 2, H * 65], f8, tag="vA", bufs=2,
                             name=f"vA_{q}")
                cbT = sp.tile([128, 4, H], f32, tag="cbT", bufs=2, name=f"cbT_{q}")
                for jtp in range(2):
                    for ko in range(2):
                        ev = vA[:, jtp, ko, :].rearrange("p (h u) -> p h u", u=65)
                        nc.vector.memset(ev[:, :, 64:65], 1.0)
                for jt in range(4):
                    jb = slice(q * R + jt * 128, q * R + jt * 128 + 128)
                    psv = [pp.tile([128, R], f32, tag="A", bufs=5,
                                   name=f"vps_{q}_{jt}_{half}")
                           for half in range(2)]
                    pscb = pp.tile([128, H], f32, tag="C", bufs=3,
                                   name=f"cbps_{q}_{jt}")
                    for dg in range(4):
                        for half in range(2):
                            nc.tensor.matmul(psv[half], xt8[:, dg, :, jb],
                                             wv8[:, dg, :, half * 512:(half + 1) * 512],
                                             start=(dg == 0), stop=(dg == 3),
                                             perf_mode=DR)
                        nc.tensor.matmul(pscb, xt8[:, dg, :, jb], wcb8[:, dg, :, :],
                                         start=(dg == 0), stop=(dg == 3),
                                         perf_mode=DR)
                    ev = vA[:, jt // 2, jt % 2, :].rearrange("p (h u) -> p h u", u=65)
                    for half in range(2):
                        nc.vector.tensor_copy(
                            out=ev[:, half * 8:(half + 1) * 8, 0:64],
                            in_=psv[half].rearrange("p (h u) -> p h u", u=64))
                    nc.vector.tensor_scalar(out=cbT[:, jt, :], in0=pscb,
                                            scalar1=1.0 / 16.0, scalar2=None,
                                            op0=ALU.mult)

                if q == 0:
                    # remaining qm (DVE), after quarter-0 casts in program order
                    for h in range(4, H):
                        for g in range(4):
                            for ko in range(2):
                                nc.vector.tensor_scalar_mul(
                                    qm[:, h, g, ko, :], qT[:, 2 * g + ko, :],
                                    mt_sb[:, 2 * g + ko, h:h + 1])

                # attention: head groups of 4
                for hg in range(4):
                    prs = []
                    for hh in range(4):
                        pr = sp.tile([128, 2, 2, R], f8, tag="pr", bufs=4,
                                     name=f"pr_{q}_{hg}_{hh}")
                        prs.append(pr)
                    for jt in range(4):
                        pss = []
                        for hh in range(4):
                            pss.append(pp.tile([128, R], f32, tag="A", bufs=5,
                                               name=f"sps_{q}_{hg}_{jt}_{hh}"))
                        for g in range(4):
                            for hh in range(4):
                                h = hg * 4 + hh
                                nc.tensor.matmul(
                                    pss[hh], kT8[:, g, :, jt * 128:(jt + 1) * 128],
                                    qm[:, h, g, :, :],
                                    start=(g == 0), stop=(g == 3), perf_mode=DR)
                        for hh in range(4):
                            h = hg * 4 + hh
                            nc.scalar.activation(
                                out=prs[hh][:, jt // 2, jt % 2, :], in_=pss[hh],
                                func=AF.Exp, bias=cbT[:, jt, h:h + 1],
                                scale=1.0 / 1024.0)
                    for hh in range(4):
                        h = hg * 4 + hh
                        cps = pp.tile([65, R], f32, tag="C", bufs=3,
                                      name=f"cps_{q}_{hg}_{hh}")
                        for jtp in range(2):
                            nc.tensor.matmul(cps, vA[:, jtp, :, h * 65:h * 65 + 65],
                                             prs[hh][:, jtp, :, :],
                                             start=(jtp == 0), stop=(jtp == 1),
                                             perf_mode=DR)
                        if q == 0:
                            nc.vector.tensor_copy(out=ctxu[:, h, :], in_=cps)
                        else:
                            nc.vector.tensor_tensor(out=ctxu[:, h, :], in0=cps,
                                                    in1=ctxu[:, h, :], op=ALU.add)

                    if q == NQ - 1:
                        # den -> reciprocal -> normalized fp8 ctx, per head
                        # group, overlapping the remaining score matmuls
                        h0 = hg * 4
                        nc.sync.dma_start(out=den_dram[:, h0:h0 + 4, :],
                                          in_=ctxu[64:65, h0:h0 + 4, :])
                        dl = sp.tile([4, R], bf16, tag="dl", bufs=2,
                                     name=f"dl_{hg}")
                        dsrc = bass.AP(tensor=den_dram.tensor,
                                       offset=den_dram[0:1, h0:h0 + 4, :].offset,
                                       ap=[[R, 4], [1, R]])
                        nc.sync.dma_start(out=dl, in_=dsrc)
                        rec4 = sp.tile([4, R], f32, tag="rec", bufs=2,
                                       name=f"rec_{hg}")
                        nc.vector.reciprocal(out=rec4, in_=dl)
                        nc.sync.dma_start(out=rec_dram[h0:h0 + 4, :], in_=rec4)
                        for hh in range(4):
                            h = h0 + hh
                            rb = sp.tile([64, R], f32, tag="rb", bufs=2,
                                         name=f"rb_{h}")
                            src = bass.AP(tensor=rec_dram.tensor,
                                          offset=rec_dram[h:h + 1, :].offset,
                                          ap=[[0, 64], [1, R]])
                            nc.sync.dma_start(out=rb, in_=src)
                            nc.vector.tensor_tensor(
                                out=ctxn[:, h // 2, h % 2, :],
                                in0=ctxu[0:64, h, :], in1=rb, op=ALU.mult)

            # dense (psum = 256*(ctx@Wd)) + residual (xr2 = 256*(x+bd')) + LN
            for ic in range(4):
                res = sp.tile([128, D], f32, tag="res", bufs=1, name=f"res_{ic}")
                xrc = sp.tile([128, D], f32, tag="xrc", bufs=1, name=f"xrc_{ic}")
                nc.sync.dma_start(out=xrc, in_=xr2_d[ic * 128:(ic + 1) * 128, :])
                for oh in range(2):
                    ps = pp.tile([128, 512], f32, tag="A", bufs=5,
                                 name=f"dps_{ic}_{oh}")
                    for s in range(8):
                        nc.tensor.matmul(ps, ctxn[:, s, :, ic * 128:(ic + 1) * 128],
                                         wd8[:, s, :, oh * 512:(oh + 1) * 512],
                                         start=(s == 0), stop=(s == 7),
                                         perf_mode=DR)
                    nc.vector.tensor_tensor(
                        out=res[:, oh * 512:(oh + 1) * 512], in0=ps,
                        in1=xrc[:, oh * 512:(oh + 1) * 512], op=ALU.add)
                stats = sp.tile([128, 2, nc.vector.BN_STATS_DIM], f32, tag="stats",
                                bufs=2, name=f"stats_{ic}")
                for g in range(2):
                    nc.vector.bn_stats(out=stats[:, g, :],
                                       in_=res[:, g * 512:(g + 1) * 512])
                mv = sp.tile([128, nc.vector.BN_AGGR_DIM], f32, tag="mv", bufs=2,
                             name=f"mv_{ic}")
                nc.vector.bn_aggr(out=mv, in_=stats)
                rstd = sp.tile([128, 1], f32, tag="rstd", bufs=2, name=f"rstd_{ic}")
                nc.scalar.activation(out=rstd, in_=mv[:, 1:2], func=AF.Sqrt,
                                     bias=epsT, scale=1.0)
                nc.vector.reciprocal(out=rstd, in_=rstd)
                lnA = sp.tile([128, D], f32, tag="lnA", bufs=2, name=f"lnA_{ic}")
                nc.vector.tensor_scalar(out=lnA, in0=res, scalar1=mv[:, 0:1],
                                        scalar2=rstd, op0=ALU.subtract,
                                        op1=ALU.mult)
                nc.vector.tensor_tensor(out=res, in0=lnA, in1=gamB, op=ALU.mult)
                nc.vector.tensor_tensor(out=lnA, in0=res, in1=betB, op=ALU.add)
                nc.sync.dma_start(out=out_d[ic * 128:(ic + 1) * 128, :], in_=lnA)

    nc.compile()
    return nc


def _arr8(mat, scale):
    """[Drows, C] f32 -> [128, 4, 2, C] fp8 with rows d = dg*256 + ko*128 + p."""
    import ml_dtypes
    a = (mat * scale).astype(ml_dtypes.float8_e4m3)
    C = a.shape[1]
    return np.ascontiguousarray(a.reshape(4, 2, 128, C).transpose(2, 0, 1, 3))


def _prep_in_maps(inputs):
    import ml_dtypes
    f = np.float32
    x = np.ascontiguousarray(np.asarray(inputs["hidden_states"], f))
    Wq = np.asarray(inputs["Wq"], f)
    Wk = np.asarray(inputs["Wk"], f)
    Wcb = np.asarray(inputs["Wcb"], f)
    Wv = np.asarray(inputs["Wv"], f)
    bv = np.asarray(inputs["bv"], f)
    mixing = np.asarray(inputs["mixing"], f)
    Wd = np.asarray(inputs["Wd"], f)
    bd = np.asarray(inputs["bd"], f)
    gamma = np.asarray(inputs["gamma"], f)
    beta = np.asarray(inputs["beta"], f)

    bde = (bd + Wd @ bv).astype(f)
    mt = np.ascontiguousarray(
        (mixing.T * 0.5).reshape(8, 128, H).transpose(1, 0, 2)).astype(f)
    shared = {
        "wq8": _arr8(Wq.T, 16.0),
        "wk8": _arr8(Wk.T, 16.0),
        "wv8": _arr8(Wv.T, 16.0),
        # dense stationary convention: v = (2s+ko)*64 + u -> [u, s, ko, o]
        "wd8": np.ascontiguousarray(
            (Wd.T * 16.0).astype(ml_dtypes.float8_e4m3)
            .reshape(8, 2, 64, D).transpose(2, 0, 1, 3)),
        "wcb8": _arr8(Wcb.T, 2.0),
        "mt": mt,
        "gamma2": np.ascontiguousarray(gamma[None, :]).astype(ml_dtypes.bfloat16),
        "beta2": np.ascontiguousarray(beta[None, :]).astype(ml_dtypes.bfloat16),
    }
    xb8 = [np.asarray(x[b].T, ml_dtypes.float8_e4m3) for b in range(B)]
    in_maps = []
    for c in range(8):
        b, rb = divmod(c, 4)
        r0 = rb * R
        cols = np.r_[r0:r0 + R, 0:r0, r0 + R:S]
        xp = xb8[b][:, cols]
        xt8 = np.ascontiguousarray(xp.reshape(4, 2, 128, S).transpose(2, 0, 1, 3))
        xr2 = np.ascontiguousarray(256.0 * (x[b, r0:r0 + R] + bde[None, :]))
        in_maps.append({"xt8": xt8, "xr2": xr2, **shared})
    return in_maps


def _gather(results):
    out = np.empty((B, S, D), np.float32)
    for c in range(8):
        b, rb = divmod(c, 4)
        out[b, rb * R:(rb + 1) * R] = results[c]["out"]
    return out


def kernel(**inputs):
    from concourse.bass_utils import run_bass_kernel_spmd

    if "nc" not in _CACHE:
        _CACHE["nc"] = _build()
    nc = _CACHE["nc"]
    in_maps = _prep_in_maps(inputs)
    res = run_bass_kernel_spmd(nc, in_maps, core_ids=list(range(8)))
    return (_gather(res.results),)
